# revision 16
# baseline (speedup 1.0000x reference)
"""Trainium2 Bass kernel for nn_Attention_TopK (ViT attention + top-k token pruning).

Sharding: data-parallel over batch B=64 across 8 NeuronCores (8 batches/core).
Bulk path (out): bf16 matmuls, fp32 PSUM accumulate.
CLS/top-k path (idx): exact fp32 side computation via the u-factorization
  S_cls[h,j] = (Wk[h]^T q_cls) . x_j  -> fp32 softmax -> mean over heads ->
  rank via pairwise-comparison matrix -> permutation inversion by matmul.
"""

import numpy as np
import ml_dtypes

import concourse.bass as bass
import concourse.bacc as bacc
import concourse.mybir as mybir
import concourse.tile as tile
from concourse.bass_utils import run_bass_kernel_spmd

B, N, C = 64, 197, 768
H, HD = 12, 64
NCORES = 8
BPC = B // NCORES          # 8 batches per core
KTOP = 137
NT = 196                   # tokens ranked (tokens 1..196)
SCALE = 1.0 / 8.0          # hd^-0.5
KC = C // 128              # 6 contraction chunks
MC3 = 3 * C // 128         # 18 output chunks for qkv
TOKC = [(0, 128), (128, 69)]   # (offset, size) token chunks of 197
PCH = [(0, 128), (128, 68)]    # partition chunks of 196

f32 = mybir.dt.float32
bf16 = mybir.dt.bfloat16
i32 = mybir.dt.int32
AX = mybir.AxisListType.X
OP = mybir.AluOpType
ACT = mybir.ActivationFunctionType

_CACHE = {}


def _build_nc():
    nc = bacc.Bacc("TRN2", target_bir_lowering=False, debug=False,
                   num_devices=NCORES)

    d = {}
    def din(name, shape, dt):
        d[name] = nc.dram_tensor(name, shape, dt, kind="ExternalInput").ap()
    def dout(name, shape, dt):
        d[name] = nc.dram_tensor(name, shape, dt, kind="ExternalOutput").ap()

    din("xT_bf", [BPC, C, N], bf16)       # x^T per batch, bf16
    din("xT_f32", [BPC, C, N], f32)       # x^T per batch, fp32 (CLS path)
    din("wqkvT_bf", [C, 3 * C], bf16)     # qkv_w^T
    din("wqT_f32", [C, C], f32)           # (Wq)^T = qkv_w[:768].T
    din("wk_nat", [C, C], f32)            # Wk = qkv_w[768:1536]
    din("wprojT_bf", [C, C], bf16)        # proj_w^T
    din("bias_rep", [128, C], f32)        # proj_b replicated over partitions
    din("ident_bf", [128, 128], bf16)
    din("ident_f32", [128, 128], f32)
    din("jlt", [NT, NT], f32)             # jlt[i,j] = 1.0 if j<i else 0
    din("piota", [128, KTOP], f32)        # each row = arange(137)
    din("ivec", [NT, 1], f32)             # arange(196) column
    din("ones12", [H, 1], f32)
    din("ones128", [1, 128], f32)
    dout("out", [BPC, N, C], f32)
    dout("idx", [BPC, KTOP], i32)

    with tile.TileContext(nc) as tc:
        _kernel_body(tc, nc, d)
    nc.compile()
    return nc


def _kernel_body(tc, nc, d):
    import contextlib
    ctx = contextlib.ExitStack()
    consts = ctx.enter_context(tc.tile_pool(name="consts", bufs=1))
    persist = ctx.enter_context(tc.tile_pool(name="persist", bufs=1))
    work = ctx.enter_context(tc.tile_pool(name="work", bufs=2))
    small = ctx.enter_context(tc.tile_pool(name="small", bufs=3))
    pp = ctx.enter_context(tc.tile_pool(name="pp", bufs=1, space="PSUM"))
    pp2 = pp

    dma = nc.gpsimd.dma_start

    # ---- load weights & constants into SBUF ----
    wqkv_sb = consts.tile([128, KC * 3 * C], bf16)
    wproj_sb = consts.tile([128, KC * C], bf16)
    wq32_sb = consts.tile([128, KC * C], f32)
    wk_sb = consts.tile([128, KC * C], f32)
    bias_sb = consts.tile([128, C], f32)
    idb_sb = consts.tile([128, 128], bf16)
    idf_sb = consts.tile([128, 128], f32)
    jlt0_sb = consts.tile([128, NT], f32)
    jlt1_sb = consts.tile([68, NT], f32)
    pio_sb = consts.tile([128, KTOP], f32)
    iv0_sb = consts.tile([128, 1], f32)
    iv1_sb = consts.tile([68, 1], f32)
    on12_sb = consts.tile([H, 1], f32)
    on128_sb = consts.tile([1, 128], f32)

    for k in range(KC):
        dma(wqkv_sb[:, k * 3 * C:(k + 1) * 3 * C], d["wqkvT_bf"][k * 128:(k + 1) * 128, :])
        dma(wproj_sb[:, k * C:(k + 1) * C], d["wprojT_bf"][k * 128:(k + 1) * 128, :])
        dma(wq32_sb[:, k * C:(k + 1) * C], d["wqT_f32"][k * 128:(k + 1) * 128, :])
        dma(wk_sb[:, k * C:(k + 1) * C], d["wk_nat"][k * 128:(k + 1) * 128, :])
    dma(bias_sb[:], d["bias_rep"][:])
    dma(idb_sb[:], d["ident_bf"][:])
    dma(idf_sb[:], d["ident_f32"][:])
    dma(jlt0_sb[:], d["jlt"][0:128, :])
    dma(jlt1_sb[:], d["jlt"][128:NT, :])
    dma(pio_sb[:], d["piota"][:])
    dma(iv0_sb[:], d["ivec"][0:128, :])
    dma(iv1_sb[:], d["ivec"][128:NT, :])
    dma(on12_sb[:], d["ones12"][:])
    dma(on128_sb[:], d["ones128"][:])

    # fp32 x^T for all batches, retained for the CLS pass
    xt32_sb = persist.tile([128, KC * BPC * N], f32)
    qc_sb = persist.tile([128, KC * BPC], f32)       # q_cls per k-chunk x batch
    u_sb = persist.tile([128, KC * BPC * H], f32)    # u vectors, col = b*12+h

    xt32_v = xt32_sb.rearrange("p (k b n) -> p k b n", k=KC, b=BPC, n=N)
    for b in range(BPC):
        nc.gpsimd.dma_start(xt32_v[:, :, b, :],
                            d["xT_f32"][b].rearrange("(k p) n -> p k n", p=128))

    # touch ops: make each compute engine observe the load-DMA semaphore once
    touch_ps = pp.tile([1, 1], f32, tag="cls_ps", bufs=2)
    nc.tensor.matmul(touch_ps[:], on12_sb[0:1, :], on12_sb[0:1, :],
                     start=True, stop=True)
    touch_sb = small.tile([1, 2], f32, tag="touch")
    nc.vector.tensor_copy(touch_sb[0:1, 0:1], bias_sb[0:1, 0:1])
    nc.scalar.mul(touch_sb[0:1, 1:2], wk_sb[0:1, 0:1], 1.0)

    # =================== per-batch bulk path ===================
    for b in range(BPC):
        xbf_sb = work.tile([128, KC * N], bf16, tag="xbf", bufs=8)
        nc.gpsimd.dma_start(xbf_sb.rearrange("p (k n) -> p k n", n=N),
                            d["xT_bf"][b].rearrange("(k p) n -> p k n", p=128))

        # qkv^T = qkv_w @ x^T : [2304, 197] bf16
        qkvT = work.tile([128, MC3 * N], bf16, tag="qkvT")
        for m in range(MC3):
            q_ps = pp.tile([128, N], f32, tag="mm_ps", bufs=3)
            for k in range(KC):
                nc.tensor.matmul(q_ps[:],
                                 wqkv_sb[:, k * 3 * C + m * 128: k * 3 * C + (m + 1) * 128],
                                 xbf_sb[:, k * N:(k + 1) * N],
                                 start=(k == 0), stop=(k == KC - 1))
            nc.vector.tensor_copy(qkvT[:, m * N:(m + 1) * N], q_ps[:])

        # attention per head
        otb = work.tile([128, KC * N], bf16, tag="otb")   # O^T bf16 [768, 197]
        for h in range(H):
            mq, off = h // 2, (h % 2) * 64
            mk, mv = 6 + h // 2, 12 + h // 2
            at0 = small.tile([128, N], bf16, tag="at0")
            at1 = small.tile([69, N], bf16, tag="at1")
            for qc, (qoff, mcs) in enumerate(TOKC):
                s_ps = pp.tile([128, N], f32, tag="mm_ps", bufs=3)
                nc.tensor.matmul(s_ps[0:mcs, :],
                                 qkvT[off:off + 64, mq * N + qoff: mq * N + qoff + mcs],
                                 qkvT[off:off + 64, mk * N: mk * N + N],
                                 start=True, stop=True)
                mx = small.tile([128, 1], f32, tag="mx")
                nmx = small.tile([128, 1], f32, tag="nmx")
                ssum = small.tile([128, 1], f32, tag="ssum")
                rec = small.tile([128, 1], f32, tag="rec")
                a32 = small.tile([128, N], f32, tag="a32")
                abf = small.tile([128, N], bf16, tag="abf")
                nc.vector.tensor_reduce(mx[0:mcs], s_ps[0:mcs, :], axis=AX, op=OP.max)
                nc.scalar.mul(nmx[0:mcs], mx[0:mcs], -SCALE)
                nc.scalar.activation(a32[0:mcs, :], s_ps[0:mcs, :], ACT.Exp,
                                     bias=nmx[0:mcs], scale=SCALE,
                                     accum_out=ssum[0:mcs])
                nc.vector.reciprocal(rec[0:mcs], ssum[0:mcs])
                nc.vector.tensor_scalar_mul(abf[0:mcs, :], a32[0:mcs, :], rec[0:mcs])
                # transpose A chunk -> at0/at1
                for cc, (koff, kn) in enumerate(TOKC):
                    t_ps = pp2.tile([128, 128], bf16, tag="tr_ps", bufs=2)
                    nc.tensor.transpose(t_ps[0:kn, 0:mcs],
                                        abf[0:mcs, koff:koff + kn],
                                        idb_sb[0:mcs, 0:mcs])
                    dst = at0 if cc == 0 else at1
                    nc.vector.tensor_copy(dst[0:kn, qoff:qoff + mcs], t_ps[0:kn, 0:mcs])
            # transpose v^T [64,197] -> v natural chunks
            vn0 = small.tile([128, 64], bf16, tag="vn0")
            vn1 = small.tile([69, 64], bf16, tag="vn1")
            for cc, (koff, kn) in enumerate(TOKC):
                v_ps = pp2.tile([128, 64], bf16, tag="tr_ps", bufs=2)
                nc.tensor.transpose(v_ps[0:kn, 0:64],
                                    qkvT[off:off + 64, mv * N + koff: mv * N + koff + kn],
                                    idb_sb[off:off + 64, off:off + 64])
                dst = vn0 if cc == 0 else vn1
                nc.vector.tensor_copy(dst[0:kn, :], v_ps[0:kn, 0:64])
            # O^T_h = v^T A^T : [64, 197]
            o_ps = pp.tile([64, N], f32, tag="mm_ps", bufs=3)
            nc.tensor.matmul(o_ps[:], vn0[:, :], at0[:, :], start=True, stop=False)
            nc.tensor.matmul(o_ps[:], vn1[0:69, :], at1[0:69, :], start=False, stop=True)
            nc.vector.tensor_copy(otb[off:off + 64, (h // 2) * N:(h // 2) * N + N], o_ps[:])

        # proj: y = O @ proj_w^T + b  -> [197, 768]
        y_sb = work.tile([128, C], f32, tag="y_sb", bufs=8)
        for qc, (qoff, mcs) in enumerate(TOKC):
            for ns in range(2):
                y_ps = pp.tile([128, 384], f32, tag="mm_ps", bufs=3)
                for k in range(KC):
                    nc.tensor.matmul(y_ps[0:mcs, :],
                                     otb[:, k * N + qoff: k * N + qoff + mcs],
                                     wproj_sb[:, k * C + ns * 384: k * C + ns * 384 + 384],
                                     start=(k == 0), stop=(k == KC - 1))
                nc.vector.tensor_tensor(y_sb[0:mcs, ns * 384:(ns + 1) * 384],
                                        y_ps[0:mcs, :],
                                        bias_sb[0:mcs, ns * 384:(ns + 1) * 384],
                                        op=OP.add)
            nc.sync.dma_start(d["out"][b, qoff:qoff + mcs, :], y_sb[0:mcs, :])

    # =================== CLS / top-k path (fp32 exact) ===================
    tc.strict_bb_all_engine_barrier()
    # q_cls = Wq @ x_cls for all batches: QC [768, 8]
    for m in range(KC):
        qc_ps = pp2.tile([128, BPC], f32, tag="cls_ps", bufs=2)
        for k in range(KC):
            xcls = xt32_sb[:, k * BPC * N: (k * BPC + BPC) * N].rearrange(
                "p (b n) -> p b n", n=N)[:, :, 0]
            nc.tensor.matmul(qc_ps[:],
                             wq32_sb[:, k * C + m * 128: k * C + (m + 1) * 128],
                             xcls,
                             start=(k == 0), stop=(k == KC - 1))
        nc.vector.tensor_copy(qc_sb[:, m * BPC:(m + 1) * BPC], qc_ps[:])

    # u_{b,h} = Wk[h]^T q_cls[h slice, b]  -> u_sb col layout b*12+h per k-chunk
    uv = u_sb.rearrange("p (k b h) -> p k b h", k=KC, b=BPC, h=H)
    for h in range(H):
        rq, roff = h // 2, (h % 2) * 64
        for m in range(KC):
            u_ps = pp2.tile([128, BPC], f32, tag="cls_ps", bufs=2)
            nc.tensor.matmul(u_ps[:],
                             wk_sb[roff:roff + 64, rq * C + m * 128: rq * C + (m + 1) * 128],
                             qc_sb[roff:roff + 64, rq * BPC:(rq + 1) * BPC],
                             start=True, stop=True)
            nc.vector.tensor_copy(uv[:, m, :, h], u_ps[:])

    tc.strict_bb_all_engine_barrier()
    for b in range(BPC):
        # S_cls [12, 197] fp32
        sc_ps = pp.tile([H, N], f32, tag="cls_ps", bufs=2)
        for k in range(KC):
            nc.tensor.matmul(sc_ps[:],
                             u_sb[:, (k * BPC + b) * H: (k * BPC + b + 1) * H],
                             xt32_sb[:, (k * BPC + b) * N: (k * BPC + b + 1) * N],
                             start=(k == 0), stop=(k == KC - 1))
        mxc = small.tile([H, 1], f32, tag="mxc")
        nmxc = small.tile([H, 1], f32, tag="nmxc")
        sumc = small.tile([H, 1], f32, tag="sumc")
        ac = small.tile([H, N], f32, tag="ac")
        nc.vector.tensor_reduce(mxc[:], sc_ps[:], axis=AX, op=OP.max)
        nc.scalar.mul(nmxc[:], mxc[:], -SCALE)
        nc.scalar.activation(ac[:], sc_ps[:], ACT.Exp, bias=nmxc[:], scale=SCALE,
                             accum_out=sumc[:])
        recc = small.tile([H, 1], f32, tag="recc")
        nc.vector.reciprocal(recc[:], sumc[:])
        nc.vector.tensor_scalar_mul(ac[:], ac[:], recc[:])
        # mean over heads (scale irrelevant for ranking): [1, 196]
        mn_ps = pp2.tile([1, NT], f32, tag="cls_ps", bufs=2)
        nc.tensor.matmul(mn_ps[:], on12_sb[:], ac[:, 1:N], start=True, stop=True)
        crow = small.tile([1, NT], f32, tag="crow")
        nc.vector.tensor_copy(crow[:], mn_ps[:])
        # column version via PE transpose; broadcast via ones matmul
        col0 = small.tile([128, 1], f32, tag="col0")
        col1 = small.tile([68, 1], f32, tag="col1")
        for cc, (poff, pn) in enumerate(PCH):
            c_ps = pp2.tile([128, 1], f32, tag="cls_ps", bufs=2)
            nc.tensor.transpose(c_ps[0:pn, 0:1], crow[0:1, poff:poff + pn],
                                idf_sb[0:1, 0:1])
            dst = col0 if cc == 0 else col1
            nc.vector.tensor_copy(dst[0:pn, :], c_ps[0:pn, 0:1])
        bc_ps = pp.tile([128, NT], f32, tag="cls_ps", bufs=2)
        nc.tensor.matmul(bc_ps[:], on128_sb[:], crow[:], start=True, stop=True)
        bc_sb = small.tile([128, NT], f32, tag="bc_sb")
        nc.vector.tensor_copy(bc_sb[:], bc_ps[:])
        # rank + one-hot + index recovery
        ix_ps = pp2.tile([1, KTOP], f32, tag="cls_ps", bufs=2)
        for cc, (poff, pn) in enumerate(PCH):
            colc = col0 if cc == 0 else col1
            jltc = jlt0_sb if cc == 0 else jlt1_sb
            ivc = iv0_sb if cc == 0 else iv1_sb
            tmp = small.tile([128, NT], f32, tag="tmp")
            tmp2 = small.tile([128, NT], f32, tag="tmp2")
            rk = small.tile([128, 1], f32, tag="rk")
            e_sb = small.tile([128, KTOP], f32, tag="e_sb")
            nc.vector.scalar_tensor_tensor(tmp[0:pn, :], bc_sb[0:pn, :],
                                           colc[0:pn], jltc[0:pn, :],
                                           op0=OP.is_equal, op1=OP.mult)
            nc.vector.scalar_tensor_tensor(tmp2[0:pn, :], bc_sb[0:pn, :],
                                           colc[0:pn], tmp[0:pn, :],
                                           op0=OP.is_gt, op1=OP.add,
                                           accum_out=rk[0:pn])
            nc.vector.tensor_scalar(e_sb[0:pn, :], pio_sb[0:pn, :], rk[0:pn],
                                    None, op0=OP.is_equal)
            nc.tensor.matmul(ix_ps[:], ivc[0:pn, :], e_sb[0:pn, :],
                             start=(cc == 0), stop=(cc == 1))
        ixi = small.tile([1, KTOP], i32, tag="ixi", bufs=8)
        nc.vector.tensor_copy(ixi[:], ix_ps[:])
        nc.sync.dma_start(d["idx"][b:b + 1, :], ixi[:])

    ctx.close()


def kernel(x, qkv_w, proj_w, proj_b):
    x = np.asarray(x, np.float32)
    qkv_w = np.asarray(qkv_w, np.float32)
    proj_w = np.asarray(proj_w, np.float32)
    proj_b = np.asarray(proj_b, np.float32)

    if "nc" not in _CACHE:
        _CACHE["nc"] = _build_nc()
    nc = _CACHE["nc"]

    xT = np.ascontiguousarray(x.transpose(0, 2, 1))            # [B, C, N]
    xT_bf = xT.astype(ml_dtypes.bfloat16)
    wqkvT = np.ascontiguousarray(qkv_w.T)                      # [C, 3C]
    wqkvT_bf = wqkvT.astype(ml_dtypes.bfloat16)
    wqT_f32 = np.ascontiguousarray(qkv_w[0:C, :].T)            # [C, C]
    wk_nat = np.ascontiguousarray(qkv_w[C:2 * C, :])           # [C, C]
    wprojT_bf = np.ascontiguousarray(proj_w.T).astype(ml_dtypes.bfloat16)
    bias_rep = np.ascontiguousarray(np.broadcast_to(proj_b, (128, C))).astype(np.float32)
    ident = np.eye(128, dtype=np.float32)
    jlt = (np.arange(NT)[None, :] < np.arange(NT)[:, None]).astype(np.float32)
    piota = np.ascontiguousarray(np.broadcast_to(np.arange(KTOP, dtype=np.float32), (128, KTOP)))
    ivec = np.arange(NT, dtype=np.float32)[:, None].copy()
    ones12 = np.ones((H, 1), np.float32)
    ones128 = np.ones((1, 128), np.float32)

    in_maps = []
    for c in range(NCORES):
        sl = slice(c * BPC, (c + 1) * BPC)
        in_maps.append({
            "xT_bf": np.ascontiguousarray(xT_bf[sl]),
            "xT_f32": np.ascontiguousarray(xT[sl]),
            "wqkvT_bf": wqkvT_bf,
            "wqT_f32": wqT_f32,
            "wk_nat": wk_nat,
            "wprojT_bf": wprojT_bf,
            "bias_rep": bias_rep,
            "ident_bf": ident.astype(ml_dtypes.bfloat16),
            "ident_f32": ident,
            "jlt": jlt,
            "piota": piota,
            "ivec": ivec,
            "ones12": ones12,
            "ones128": ones128,
        })

    _CACHE["in_maps"] = in_maps
    rb = run_bass_kernel_spmd(nc, in_maps, core_ids=list(range(NCORES)))
    res = rb.results

    out = np.empty((B, N, C), np.float32)
    idx = np.empty((B, KTOP), np.int32)
    for c in range(NCORES):
        out[c * BPC:(c + 1) * BPC] = res[c]["out"]
        idx[c * BPC:(c + 1) * BPC] = res[c]["idx"]
    index = np.ascontiguousarray(np.broadcast_to(idx[:, :, None], (B, KTOP, C))).astype(np.int32)
    return (out, index, idx)
